# revision 1
# baseline (speedup 1.0000x reference)
"""Trainium2 Bass kernel for nn_Decoder (latent MLP -> GRU scan -> per-step MLP).

Strategy: pure data-parallel over batch (4096 -> 8 x 512), weights replicated.
On-chip layout is fully transposed (feature dim on partitions, batch on free
dim) so GRU gate biases are per-partition ACT bias operands and gi/gh gate
pre-activations accumulate in PSUM. Matmul inputs are bf16 (fp32 PSUM
accumulation); the final per-step matmul is computed batch-major so
predictions land in [B, A] orientation without transposes.

Self-contained: hardcodes shapes from the problem spec.
"""
import sys
sys.path.insert(0, "/opt/trn_rl_repo")
from contextlib import ExitStack

import numpy as np
import ml_dtypes

import concourse.bacc as bacc
import concourse.mybir as mybir
from concourse import tile
from concourse import bass_utils

BF16 = ml_dtypes.bfloat16
BF = mybir.dt.bfloat16
F32 = mybir.dt.float32
AF = mybir.ActivationFunctionType
ALU = mybir.AluOpType

N_CORES = 8
B, LAT, H, A, L = 4096, 256, 512, 64, 128
BOS = 0
T = L - 1          # recurrence steps
BL = B // N_CORES  # per-core batch
KH = H // 128


def _build(steps=T, n_cores=N_CORES, reps=1, timing_iters=None, unroll=8,
           variant="full"):
    """Always declares the full-size DRAM interface (xT[T], y[:, L]); `steps`
    bounds the recurrence so short builds are wall-clock comparable.

    timing_iters: if set, wraps `unroll` statically-addressed step bodies in a
    hardware For_i loop executed timing_iters//unroll times (numerics garbage,
    per-step work identical) — used only to measure per-step device time."""
    nc = bacc.Bacc("TRN2", target_bir_lowering=False, debug=False,
                   num_devices=n_cores)

    d = {}
    def din(name, shape, dt=BF):
        d[name] = nc.dram_tensor(name, list(shape), dt, kind="ExternalInput").ap()

    din("latentT", [LAT, BL])
    din("xT", [T, A, BL])
    din("WhhT", [H, 3 * H])
    din("WihT", [A, 3 * H])
    din("Wm1T", [H, H])
    din("Wm2T", [H, A])
    din("Wm3T", [A, A])
    din("Wd1T", [LAT, H])
    din("Wd2T", [H, H])
    din("Wd3T", [H, H])
    din("b_rz", [2 * H], F32)
    din("b_inn", [H], F32)
    din("b_hnn", [H], F32)
    din("bm1", [H], F32)
    din("bm2", [A], F32)
    din("bm3b", [128, A], F32)
    din("bd1", [H], F32)
    din("bd2", [H], F32)
    din("bd3", [H], F32)
    y = nc.dram_tensor("y", [BL, L, A], F32, kind="ExternalOutput").ap()

    with tile.TileContext(nc) as tc, ExitStack() as ctx:
        cst = ctx.enter_context(tc.tile_pool(name="const", bufs=1))
        wrk = ctx.enter_context(tc.tile_pool(name="work", bufs=2))
        hpool = ctx.enter_context(tc.tile_pool(name="hp", bufs=4))
        ps = ctx.enter_context(tc.tile_pool(name="ps", bufs=7, space="PSUM"))
        psy = ctx.enter_context(tc.tile_pool(name="psy", bufs=1, space="PSUM"))

        def const_tile(shape, dt, tag, src):
            t = cst.tile(list(shape), dt, tag=tag, name=tag)
            nc.sync.dma_start(t[:], src)
            return t

        whh = [const_tile([128, 3 * H], BF, f"whh{k}",
                          d["WhhT"][k * 128:(k + 1) * 128, :]) for k in range(KH)]
        wih = const_tile([A, 3 * H], BF, "wih", d["WihT"][:])
        wm1 = [const_tile([128, H], BF, f"wm1{k}",
                          d["Wm1T"][k * 128:(k + 1) * 128, :]) for k in range(KH)]
        wm2 = [const_tile([128, A], BF, f"wm2{k}",
                          d["Wm2T"][k * 128:(k + 1) * 128, :]) for k in range(KH)]
        wm3 = const_tile([A, A], BF, "wm3", d["Wm3T"][:])
        wd1 = [const_tile([128, H], BF, f"wd1{k}",
                          d["Wd1T"][k * 128:(k + 1) * 128, :]) for k in range(2)]
        wd2 = [const_tile([128, H], BF, f"wd2{k}",
                          d["Wd2T"][k * 128:(k + 1) * 128, :]) for k in range(KH)]
        wd3 = [const_tile([128, H], BF, f"wd3{k}",
                          d["Wd3T"][k * 128:(k + 1) * 128, :]) for k in range(KH)]

        def bias_tiles(name, n, tag):
            return [const_tile([128, 1], F32, f"{tag}{j}",
                               d[name][j * 128:(j + 1) * 128, None])
                    for j in range(n)]

        brz = bias_tiles("b_rz", 8, "brz")
        binn = bias_tiles("b_inn", KH, "binn")
        bhnn = bias_tiles("b_hnn", KH, "bhnn")
        bm1 = bias_tiles("bm1", KH, "bm1")
        bm2 = const_tile([A, 1], F32, "bm2", d["bm2"][:, None])
        bm3b = const_tile([128, A], F32, "bm3b", d["bm3b"][:])
        bd1 = bias_tiles("bd1", KH, "bd1")
        bd2 = bias_tiles("bd2", KH, "bd2")
        bd3 = bias_tiles("bd3", KH, "bd3")

        lat = [const_tile([128, BL], BF, f"lat{k}",
                          d["latentT"][k * 128:(k + 1) * 128, :]) for k in range(2)]

        def mlp_layer(w_tiles, rhs_tiles, bias, act, out_tag):
            outs = []
            for m in range(KH):
                acc = ps.tile([128, BL], F32, tag="ps", name="ps")
                nk = len(rhs_tiles)
                for k in range(nk):
                    nc.tensor.matmul(
                        acc[:], w_tiles[k][:, m * 128:(m + 1) * 128],
                        rhs_tiles[k][:], start=(k == 0), stop=(k == nk - 1))
                o = hpool.tile([128, BL], BF, tag=f"{out_tag}{m}",
                               name=f"{out_tag}{m}")
                nc.scalar.activation(o[:], acc[:], act, bias=bias[m][:])
                outs.append(o)
            return outs

        h1 = mlp_layer(wd1, lat, bd1, AF.Tanh, "h1")
        h2 = mlp_layer(wd2, h1, bd2, AF.Tanh, "h2")
        hb = mlp_layer(wd3, h2, bd3, AF.Identity, "hb")

        # hist[t] = h tiles of step t (init state = hist[-1]); pred for step t
        # is emitted 2 steps later so its matmuls fill the PE stall while the
        # h(t-1) elementwise tail completes (PE is in-order; only work that
        # depends on older state can hide the recurrence tail).
        state = {"hist": {-1: hb}, "ystage": None}
        # variant flags (dev-only timing decomposition; graded path = "full")
        want_gates = variant in ("full", "nopred")
        want_pred = variant in ("full", "mmpred")
        want_mm = variant != "eltonly"

        def gi_mm(acc, m, xt, stop):
            nc.tensor.matmul(acc[:], wih[:, m * 128:(m + 1) * 128],
                             xt[:], start=True, stop=stop)

        def emit_gates(t):
            hb = state["hist"][t - 1]
            xt = wrk.tile([A, BL], BF, tag="xt", name="xt")
            nc.sync.dma_start(xt[:], d["xT"][t])

            if not want_mm:
                state["hist"][t] = hb
                return

            # n-gate gh part first: starts the long DVE/ACT chain earliest
            hn_ps = []
            for j in range(KH):
                m = 8 + j
                hn = ps.tile([128, BL], F32, tag="ps", name="ps")
                for k in range(KH):
                    nc.tensor.matmul(
                        hn[:], whh[k][:, m * 128:(m + 1) * 128],
                        hb[k][:], start=(k == 0), stop=(k == KH - 1))
                hn_ps.append(hn)

            r = []
            for m0 in (0, 2):
                accs = []
                for m in (m0, m0 + 1):
                    acc = ps.tile([128, BL], F32, tag="ps", name="ps")
                    gi_mm(acc, m, xt, stop=False)
                    accs.append(acc)
                for i, m in enumerate((m0, m0 + 1)):
                    for k in range(KH):
                        nc.tensor.matmul(
                            accs[i][:], whh[k][:, m * 128:(m + 1) * 128],
                            hb[k][:], start=False, stop=(k == KH - 1))
                if want_gates:
                    for i, m in enumerate((m0, m0 + 1)):
                        g = wrk.tile([128, BL], BF, tag=f"rz{m}",
                                     name=f"rz{m}")
                        nc.scalar.activation(g[:], accs[i][:], AF.Sigmoid,
                                             bias=brz[m][:])
                        r.append(g)

            # n-gate chain: rhn -> s -> tanh -> d   (z matmuls run under this)
            n_list, d_list = [], []
            for j0 in (0, 2):
                inns = []
                for j in (j0, j0 + 1):
                    inn = ps.tile([128, BL], F32, tag="ps", name="ps")
                    gi_mm(inn, 8 + j, xt, stop=True)
                    inns.append(inn)
                if not want_gates:
                    continue
                for i, j in enumerate((j0, j0 + 1)):
                    rhn = wrk.tile([128, BL], F32, tag="rhn", name="rhn")
                    nc.vector.scalar_tensor_tensor(
                        rhn[:], hn_ps[j][:], bhnn[j][:], r[j][:],
                        op0=ALU.add, op1=ALU.mult)
                    s = wrk.tile([128, BL], F32, tag="s", name="s")
                    nc.vector.scalar_tensor_tensor(
                        s[:], inns[i][:], binn[j][:], rhn[:],
                        op0=ALU.add, op1=ALU.add)
                    n_t = wrk.tile([128, BL], BF, tag="nt", name="nt")
                    nc.scalar.activation(n_t[:], s[:], AF.Tanh)
                    d_t = wrk.tile([128, BL], BF, tag="dt", name="dt")
                    nc.gpsimd.tensor_sub(d_t[:], hb[j][:], n_t[:])
                    n_list.append(n_t)
                    d_list.append(d_t)

            # z gate last: shortest tail (sigmoid -> zd -> h_new)
            hb_new = []
            for j0 in (0, 2):
                accs = []
                for j in (j0, j0 + 1):
                    acc = ps.tile([128, BL], F32, tag="ps", name="ps")
                    gi_mm(acc, 4 + j, xt, stop=False)
                    accs.append(acc)
                for i, j in enumerate((j0, j0 + 1)):
                    for k in range(KH):
                        nc.tensor.matmul(
                            accs[i][:], whh[k][:, (4 + j) * 128:(5 + j) * 128],
                            hb[k][:], start=False, stop=(k == KH - 1))
                if not want_gates:
                    continue
                for i, j in enumerate((j0, j0 + 1)):
                    z = wrk.tile([128, BL], BF, tag=f"rz{4+j}",
                                 name=f"rz{4+j}")
                    nc.scalar.activation(z[:], accs[i][:], AF.Sigmoid,
                                         bias=brz[4 + j][:])
                    zd = wrk.tile([128, BL], BF, tag="zd", name="zd")
                    nc.vector.tensor_mul(zd[:], z[:], d_list[j][:])
                    nh = hpool.tile([128, BL], BF, tag=f"hb{j}", name=f"hb{j}")
                    nc.vector.tensor_add(nh[:], n_list[j][:], zd[:])
                    hb_new.append(nh)
            state["hist"][t] = hb_new if want_gates else hb

        def emit_pred(tp, last, force=False):
            if not want_pred or (tp < 0 and not force):
                return
            hb = (state["hist"][tp] if tp in state["hist"]
                  else state["hist"][-1])
            ystage = state["ystage"]
            pT = []
            for m in range(KH):
                acc = ps.tile([128, BL], F32, tag="ps", name="ps")
                if want_mm:
                    for k in range(KH):
                        nc.tensor.matmul(
                            acc[:], wm1[k][:, m * 128:(m + 1) * 128],
                            hb[k][:], start=(k == 0), stop=(k == KH - 1))
                p = wrk.tile([128, BL], BF, tag=f"pT{m}", name=f"pT{m}")
                nc.scalar.activation(p[:], acc[:], AF.Tanh, bias=bm1[m][:])
                pT.append(p)
            acc2 = ps.tile([A, BL], F32, tag="ps", name="ps")
            if want_mm:
                for k in range(KH):
                    nc.tensor.matmul(acc2[:], wm2[k][:], pT[k][:],
                                     start=(k == 0), stop=(k == KH - 1))
            p2 = wrk.tile([A, BL], BF, tag="p2", name="p2")
            nc.scalar.activation(p2[:], acc2[:], AF.Tanh, bias=bm2[:])

            tps = tp if tp >= 0 else tp + 8  # timing-build pseudo-slot
            o = (tps + 1) % 8
            g = (tps + 1) // 8
            if ystage is None or o == 0 or (g == 0 and o == 1):
                ystage = [wrk.tile([128, 8 * A], F32, tag=f"yst{bt}",
                                   name=f"yst{bt}") for bt in range(4)]
            for bt in range(4):
                yp = psy.tile([128, A], F32, tag="psy", name="psy")
                nc.tensor.matmul(yp[:], p2[:, bt * 128:(bt + 1) * 128],
                                 wm3[:], start=True, stop=True)
                nc.vector.tensor_add(
                    ystage[bt][:, o * A:(o + 1) * A], yp[:], bm3b[:])
            if o == 7 or last:
                lo = 1 if g == 0 else 0
                hi = o + 1
                for bt in range(4):
                    nc.sync.dma_start(
                        y[bt * 128:(bt + 1) * 128, g * 8 + lo:g * 8 + hi, :],
                        ystage[bt][:, lo * A:hi * A])
            state["ystage"] = ystage
            # h(tp) no longer needed once its pred is done
            state["hist"].pop(tp - 1, None)

        PRED_LAG = 2
        if timing_iters is None:
            for _rep in range(reps):
                for t in range(steps):
                    emit_pred(t - PRED_LAG, last=False)
                    emit_gates(t)
                for tp in range(max(steps - PRED_LAG, 0), steps):
                    emit_pred(tp, last=(tp == steps - 1))
        else:
            # timing loop: same per-step work (preds for t<LAG read init h —
            # numerics are garbage in timing builds anyway)
            with tc.For_i(0, timing_iters // unroll, 1):
                for t in range(unroll):
                    emit_pred(t - PRED_LAG, last=False, force=True)
                    emit_gates(t)

    nc.compile()
    return nc


def _make_bos():
    bos = np.full((B, A), -16.0, np.float32)
    bos[:, BOS] = 16.0
    return bos


def _make_in_maps(inputs, n_cores=N_CORES, T=T):
    bl = B // n_cores
    f32 = np.float32
    shared = {
        "WhhT": np.ascontiguousarray(np.asarray(inputs["W_hh"], f32).T).astype(BF16),
        "WihT": np.ascontiguousarray(np.asarray(inputs["W_ih"], f32).T).astype(BF16),
        "Wm1T": np.ascontiguousarray(np.asarray(inputs["Wm1"], f32).T).astype(BF16),
        "Wm2T": np.ascontiguousarray(np.asarray(inputs["Wm2"], f32).T).astype(BF16),
        "Wm3T": np.ascontiguousarray(np.asarray(inputs["Wm3"], f32).T).astype(BF16),
        "Wd1T": np.ascontiguousarray(np.asarray(inputs["Wd1"], f32).T).astype(BF16),
        "Wd2T": np.ascontiguousarray(np.asarray(inputs["Wd2"], f32).T).astype(BF16),
        "Wd3T": np.ascontiguousarray(np.asarray(inputs["Wd3"], f32).T).astype(BF16),
        "b_rz": (np.asarray(inputs["b_ih"], f32)
                 + np.asarray(inputs["b_hh"], f32))[:2 * H].astype(f32),
        "b_inn": np.asarray(inputs["b_ih"], f32)[2 * H:].astype(f32),
        "b_hnn": np.asarray(inputs["b_hh"], f32)[2 * H:].astype(f32),
        "bm1": np.asarray(inputs["bm1"], f32),
        "bm2": np.asarray(inputs["bm2"], f32),
        "bm3b": np.ascontiguousarray(
            np.broadcast_to(np.asarray(inputs["bm3"], f32), (128, A))),
        "bd1": np.asarray(inputs["bd1"], f32),
        "bd2": np.asarray(inputs["bd2"], f32),
        "bd3": np.asarray(inputs["bd3"], f32),
    }
    bos = _make_bos()
    latent = np.asarray(inputs["latent"], f32)
    target = np.asarray(inputs["target"], f32)
    in_maps = []
    for c in range(n_cores):
        sl = slice(c * bl, (c + 1) * bl)
        xT = np.empty((T, A, bl), BF16)
        xT[0] = bos[sl].T
        if T > 1:
            xT[1:] = target[sl, 1:T].transpose(1, 2, 0).astype(BF16)
        m = dict(shared)
        m["latentT"] = np.ascontiguousarray(latent[sl].T).astype(BF16)
        m["xT"] = xT
        in_maps.append(m)
    return in_maps


_NC_CACHE = {}


def _get_nc(steps=T, reps=1):
    key = (steps, reps)
    if key not in _NC_CACHE:
        _NC_CACHE[key] = _build(steps=steps, reps=reps)
    return _NC_CACHE[key]


def kernel(**inputs):
    nc = _get_nc()
    in_maps = _make_in_maps(inputs)
    res = bass_utils.run_bass_kernel_spmd(nc, in_maps,
                                          core_ids=list(range(N_CORES)))
    bl = B // N_CORES
    y = np.empty((B, L, A), np.float32)
    for c in range(N_CORES):
        y[c * bl:(c + 1) * bl] = res.results[c]["y"]
    y[:, 0, :] = _make_bos()
    return y



# revision 5
# speedup vs baseline: 1.0358x; 1.0358x over previous
"""Trainium2 Bass kernel for nn_Decoder (latent MLP -> GRU scan -> per-step MLP).

Strategy: pure data-parallel over batch (4096 -> 8 x 512), weights replicated.
Feature dim on partitions, batch on free dim. All recurrent matmuls (gi, gh,
wm1, wm2) run in fp8e4m3 with MatmulPerfMode.DoubleRow: each instruction
contracts TWO 128-row K-groups at 0.5 cycles/row (4x bf16 throughput). The
hidden state h is carried as fp8 "pair" tiles [128, 2, BL] matching the
DoubleRow ifmap layout. GRU tail: rhn = (hn+bhnn)*r is computed by DVE
in-place in the hn PSUM bank, then the x@W_ih n-gate matmul accumulates on
top (start=False), so tanh reads (gi_n + rhn) straight from PSUM and the
separate `s = inn + rhn` add disappears. d/zd run on Pool to offload DVE.

Self-contained: hardcodes shapes from the problem spec.
"""
import sys
sys.path.insert(0, "/opt/trn_rl_repo")
from contextlib import ExitStack

import numpy as np
import ml_dtypes

import concourse.bacc as bacc
import concourse.mybir as mybir
from concourse import tile
from concourse import bass_utils

BF16 = ml_dtypes.bfloat16
FP8 = ml_dtypes.float8_e4m3
BF = mybir.dt.bfloat16
F8 = mybir.dt.float8e4
F32 = mybir.dt.float32
AF = mybir.ActivationFunctionType
ALU = mybir.AluOpType
DR = mybir.MatmulPerfMode.DoubleRow

N_CORES = 8
B, LAT, H, A, L = 4096, 256, 512, 64, 128
BOS = 0
T = L - 1          # recurrence steps
BL = B // N_CORES  # per-core batch
KH = H // 128


def _build(steps=T, n_cores=N_CORES, reps=1, timing_iters=None, unroll=8,
           variant="full"):
    """Always declares the full-size DRAM interface (xT[T], y[:, L]); `steps`
    bounds the recurrence so short builds are wall-clock comparable.

    timing_iters: if set, wraps `unroll` statically-addressed step bodies in a
    hardware For_i loop executed timing_iters//unroll times (numerics garbage,
    per-step work identical) — used only to measure per-step device time."""
    nc = bacc.Bacc("TRN2", target_bir_lowering=False, debug=False,
                   num_devices=n_cores)

    d = {}
    def din(name, shape, dt=BF):
        d[name] = nc.dram_tensor(name, list(shape), dt, kind="ExternalInput").ap()

    din("latentT", [LAT, BL])
    din("xT", [T, 32, 2, BL], F8)
    din("Whh8_0", [128, 2, 3 * H], F8)
    din("Whh8_1", [128, 2, 3 * H], F8)
    din("Wih8", [32, 2, 3 * H], F8)
    din("Wm18_0", [128, 2, H], F8)
    din("Wm18_1", [128, 2, H], F8)
    din("Wm28_0", [128, 2, A], F8)
    din("Wm28_1", [128, 2, A], F8)
    din("Wm3T", [A, A])
    din("Wd1T", [LAT, H])
    din("Wd2T", [H, H])
    din("Wd3T", [H, H])
    din("b_rz", [2 * H], F32)
    din("b_inn", [H], F32)
    din("b_hnn", [H], F32)
    din("bm1", [H], F32)
    din("bm2", [A], F32)
    din("bm3b", [128, A], F32)
    din("bd1", [H], F32)
    din("bd2", [H], F32)
    din("bd3", [H], F32)
    y = nc.dram_tensor("y", [BL, L, A], F32, kind="ExternalOutput").ap()

    with tile.TileContext(nc) as tc, ExitStack() as ctx:
        cst = ctx.enter_context(tc.tile_pool(name="const", bufs=1))
        wrk = ctx.enter_context(tc.tile_pool(name="work", bufs=2))
        hpool = ctx.enter_context(tc.tile_pool(name="hp", bufs=4))
        ps = ctx.enter_context(tc.tile_pool(name="ps", bufs=7, space="PSUM"))
        psy = ctx.enter_context(tc.tile_pool(name="psy", bufs=1, space="PSUM"))

        def const_tile(shape, dt, tag, src):
            t = cst.tile(list(shape), dt, tag=tag, name=tag)
            nc.sync.dma_start(t[:], src)
            return t

        whh8 = [const_tile([128, 2, 3 * H], F8, f"whh8{p}", d[f"Whh8_{p}"][:])
                for p in range(2)]
        wih8 = const_tile([32, 2, 3 * H], F8, "wih8", d["Wih8"][:])
        wm18 = [const_tile([128, 2, H], F8, f"wm18{p}", d[f"Wm18_{p}"][:])
                for p in range(2)]
        wm28 = [const_tile([128, 2, A], F8, f"wm28{p}", d[f"Wm28_{p}"][:])
                for p in range(2)]
        wm3 = const_tile([A, A], BF, "wm3", d["Wm3T"][:])
        wd1 = [const_tile([128, H], BF, f"wd1{k}",
                          d["Wd1T"][k * 128:(k + 1) * 128, :]) for k in range(2)]
        wd2 = [const_tile([128, H], BF, f"wd2{k}",
                          d["Wd2T"][k * 128:(k + 1) * 128, :]) for k in range(KH)]
        wd3 = [const_tile([128, H], BF, f"wd3{k}",
                          d["Wd3T"][k * 128:(k + 1) * 128, :]) for k in range(KH)]

        def bias_tiles(name, n, tag):
            return [const_tile([128, 1], F32, f"{tag}{j}",
                               d[name][j * 128:(j + 1) * 128, None])
                    for j in range(n)]

        brz = bias_tiles("b_rz", 8, "brz")
        binn = bias_tiles("b_inn", KH, "binn")
        bhnn = bias_tiles("b_hnn", KH, "bhnn")
        bm1 = bias_tiles("bm1", KH, "bm1")
        bm2 = const_tile([A, 1], F32, "bm2", d["bm2"][:, None])
        bm3b = const_tile([128, A], F32, "bm3b", d["bm3b"][:])
        bd1 = bias_tiles("bd1", KH, "bd1")
        bd2 = bias_tiles("bd2", KH, "bd2")
        bd3 = bias_tiles("bd3", KH, "bd3")

        lat = [const_tile([128, BL], BF, f"lat{k}",
                          d["latentT"][k * 128:(k + 1) * 128, :]) for k in range(2)]

        def mlp_layer(w_tiles, rhs_tiles, bias, act, out_tag, outs=None):
            ret = []
            for m in range(KH):
                acc = ps.tile([128, BL], F32, tag="ps", name="ps")
                nk = len(rhs_tiles)
                for k in range(nk):
                    nc.tensor.matmul(
                        acc[:], w_tiles[k][:, m * 128:(m + 1) * 128],
                        rhs_tiles[k][:], start=(k == 0), stop=(k == nk - 1))
                if outs is None:
                    o = wrk.tile([128, BL], BF, tag=f"{out_tag}{m}",
                                 name=f"{out_tag}{m}")
                    nc.scalar.activation(o[:], acc[:], act, bias=bias[m][:])
                    ret.append(o)
                else:
                    nc.scalar.activation(outs[m], acc[:], act, bias=bias[m][:])
            return ret

        h1 = mlp_layer(wd1, lat, bd1, AF.Tanh, "h1")
        h2 = mlp_layer(wd2, h1, bd2, AF.Tanh, "h2")
        # final init layer writes fp8 pair tiles directly
        hp0 = [hpool.tile([128, 2, BL], F8, tag=f"hb{p}", name=f"hb{p}")
               for p in range(2)]
        mlp_layer(wd3, h2, bd3, AF.Identity, "hb",
                  outs=[hp0[m // 2][:, m % 2, :] for m in range(KH)])

        # hist[t] = h pair tiles of step t (init state = hist[-1]); pred for
        # step t is emitted 2 steps later so its matmuls fill the PE stall
        # while the h(t-1) elementwise tail completes.
        state = {"hist": {-1: hp0}, "ystage": None}

        def gh_mm(acc, m, hbp, start, stop):
            # DoubleRow gh: 2 instructions contract all 512 rows of h
            for p in range(2):
                nc.tensor.matmul(
                    acc[:], whh8[p][:, :, m * 128:(m + 1) * 128],
                    hbp[p][:, :, :], start=(start and p == 0),
                    stop=(stop and p == 1), perf_mode=DR)

        def gi_mm(acc, m, xt, start, stop, skip=False):
            nc.tensor.matmul(acc[:], wih8[:, :, m * 128:(m + 1) * 128],
                             xt[:, :, :], start=start, stop=stop, perf_mode=DR,
                             skip_group_check=skip)

        def emit_gates(t, force_pred=False):
            # pred(t-2) first: its matmuls are the only PE work independent
            # of h(t-1), so they fill the stall while the t-1 tail completes
            emit_pred(t - 2, last=False, force=force_pred)

            hbp = state["hist"][t - 1]
            xt = wrk.tile([32, 2, BL], F8, tag="xt", name="xt")
            nc.sync.dma_start(xt[:], d["xT"][t])

            # n-gate gh part first: starts the long DVE/ACT chain earliest
            hn_ps = []
            for j in range(KH):
                hn = ps.tile([128, BL], F32, tag="ps", name="ps")
                gh_mm(hn, 8 + j, hbp, start=True, stop=True)
                hn_ps.append(hn)

            rz = []
            for m0 in (0, 2, 4, 6):
                accs = []
                for m in (m0, m0 + 1):
                    acc = ps.tile([128, BL], F32, tag="ps", name="ps")
                    gi_mm(acc, m, xt, start=True, stop=False)
                    accs.append(acc)
                for i, m in enumerate((m0, m0 + 1)):
                    gh_mm(accs[i], m, hbp, start=False, stop=True)
                for i, m in enumerate((m0, m0 + 1)):
                    g = wrk.tile([128, BL], BF, tag=f"rz{m}", name=f"rz{m}")
                    nc.scalar.activation(g[:], accs[i][:], AF.Sigmoid,
                                         bias=brz[m][:])
                    rz.append(g)

            # n-gate chain: rhn -> (in-place PSUM) -> +gi_n -> tanh -> h'
            hb_new = [hpool.tile([128, 2, BL], F8, tag=f"hb{p}", name=f"hb{p}")
                      for p in range(2)]
            for j in range(KH):
                # rhn = (hn + bhnn) * r, written back into the same PSUM bank
                nc.vector.scalar_tensor_tensor(
                    hn_ps[j][:], hn_ps[j][:], bhnn[j][:], rz[j][:],
                    op0=ALU.add, op1=ALU.mult)
                # accumulate x @ W_ih_n on top of rhn
                gi_mm(hn_ps[j], 8 + j, xt, start=False, stop=True, skip=True)
                n_t = wrk.tile([128, BL], BF, tag=f"nt{j}", name=f"nt{j}")
                nc.scalar.activation(n_t[:], hn_ps[j][:], AF.Tanh,
                                     bias=binn[j][:])
                d_t = wrk.tile([128, BL], BF, tag=f"dt{j}", name=f"dt{j}")
                nc.gpsimd.tensor_sub(d_t[:], hbp[j // 2][:, j % 2, :], n_t[:])
                zd = wrk.tile([128, BL], BF, tag=f"zd{j}", name=f"zd{j}")
                nc.gpsimd.tensor_mul(zd[:], rz[4 + j][:], d_t[:])
                nc.vector.tensor_add(hb_new[j // 2][:, j % 2, :], n_t[:],
                                     zd[:])
            state["hist"][t] = hb_new

        def emit_pred(tp, last, force=False):
            if tp < 0 and not force:
                return
            hbp = (state["hist"][tp] if tp in state["hist"]
                   else state["hist"][-1])
            ystage = state["ystage"]
            p1p = [wrk.tile([128, 2, BL], F8, tag=f"p1p{p}", name=f"p1p{p}")
                   for p in range(2)]
            for m in range(KH):
                acc = ps.tile([128, BL], F32, tag="ps", name="ps")
                for p in range(2):
                    nc.tensor.matmul(
                        acc[:], wm18[p][:, :, m * 128:(m + 1) * 128],
                        hbp[p][:, :, :], start=(p == 0), stop=(p == 1),
                        perf_mode=DR)
                nc.scalar.activation(p1p[m // 2][:, m % 2, :], acc[:],
                                     AF.Tanh, bias=bm1[m][:])
            acc2 = ps.tile([A, BL], F32, tag="ps", name="ps")
            for p in range(2):
                nc.tensor.matmul(acc2[:], wm28[p][:, :, :], p1p[p][:, :, :],
                                 start=(p == 0), stop=(p == 1), perf_mode=DR)
            p2 = wrk.tile([A, BL], BF, tag="p2", name="p2")
            nc.scalar.activation(p2[:], acc2[:], AF.Tanh, bias=bm2[:])

            tps = tp if tp >= 0 else tp + 8  # timing-build pseudo-slot
            o = (tps + 1) % 8
            g = (tps + 1) // 8
            if ystage is None or o == 0 or (g == 0 and o == 1):
                ystage = [wrk.tile([128, 8 * A], F32, tag=f"yst{bt}",
                                   name=f"yst{bt}") for bt in range(4)]
            for bt in range(4):
                yp = psy.tile([128, A], F32, tag="psy", name="psy")
                nc.tensor.matmul(yp[:], p2[:, bt * 128:(bt + 1) * 128],
                                 wm3[:], start=True, stop=True)
                nc.vector.tensor_add(
                    ystage[bt][:, o * A:(o + 1) * A], yp[:], bm3b[:])
            if o == 7 or last:
                lo = 1 if g == 0 else 0
                hi = o + 1
                for bt in range(4):
                    nc.sync.dma_start(
                        y[bt * 128:(bt + 1) * 128, g * 8 + lo:g * 8 + hi, :],
                        ystage[bt][:, lo * A:hi * A])
            state["ystage"] = ystage
            # h(tp) no longer needed once its pred is done
            state["hist"].pop(tp - 1, None)

        PRED_LAG = 2
        if timing_iters is None:
            for _rep in range(reps):
                for t in range(steps):
                    emit_gates(t)
                for tp in range(max(steps - PRED_LAG, 0), steps):
                    emit_pred(tp, last=(tp == steps - 1))
        else:
            # timing loop: same per-step work (preds for t<LAG read init h —
            # numerics are garbage in timing builds anyway)
            with tc.For_i(0, timing_iters // unroll, 1):
                for t in range(unroll):
                    emit_gates(t, force_pred=True)

    nc.compile()
    return nc


def _make_bos():
    bos = np.full((B, A), -16.0, np.float32)
    bos[:, BOS] = 16.0
    return bos


def _pair(mT):
    """[2K, M] -> list of 2 fp8 pair tensors [128, 2, M] for DoubleRow."""
    K = mT.shape[0]
    assert K == 512
    return [np.ascontiguousarray(
        np.stack([mT[(2 * p) * 128:(2 * p + 1) * 128],
                  mT[(2 * p + 1) * 128:(2 * p + 2) * 128]], axis=1)
    ).astype(FP8) for p in range(2)]


def _make_in_maps(inputs, n_cores=N_CORES, T=T):
    bl = B // n_cores
    f32 = np.float32
    whhT = np.ascontiguousarray(np.asarray(inputs["W_hh"], f32).T)
    wm1T = np.ascontiguousarray(np.asarray(inputs["Wm1"], f32).T)
    wm2T = np.ascontiguousarray(np.asarray(inputs["Wm2"], f32).T)
    wihT = np.ascontiguousarray(np.asarray(inputs["W_ih"], f32).T)  # [64,3H]
    whh8 = _pair(whhT)
    wm18 = _pair(wm1T)
    wm28 = _pair(wm2T)
    wih8 = np.ascontiguousarray(
        wihT.reshape(2, 32, 3 * H).transpose(1, 0, 2)).astype(FP8)
    shared = {
        "Whh8_0": whh8[0], "Whh8_1": whh8[1],
        "Wm18_0": wm18[0], "Wm18_1": wm18[1],
        "Wm28_0": wm28[0], "Wm28_1": wm28[1],
        "Wih8": wih8,
        "Wm3T": np.ascontiguousarray(np.asarray(inputs["Wm3"], f32).T).astype(BF16),
        "Wd1T": np.ascontiguousarray(np.asarray(inputs["Wd1"], f32).T).astype(BF16),
        "Wd2T": np.ascontiguousarray(np.asarray(inputs["Wd2"], f32).T).astype(BF16),
        "Wd3T": np.ascontiguousarray(np.asarray(inputs["Wd3"], f32).T).astype(BF16),
        "b_rz": (np.asarray(inputs["b_ih"], f32)
                 + np.asarray(inputs["b_hh"], f32))[:2 * H].astype(f32),
        "b_inn": np.asarray(inputs["b_ih"], f32)[2 * H:].astype(f32),
        "b_hnn": np.asarray(inputs["b_hh"], f32)[2 * H:].astype(f32),
        "bm1": np.asarray(inputs["bm1"], f32),
        "bm2": np.asarray(inputs["bm2"], f32),
        "bm3b": np.ascontiguousarray(
            np.broadcast_to(np.asarray(inputs["bm3"], f32), (128, A))),
        "bd1": np.asarray(inputs["bd1"], f32),
        "bd2": np.asarray(inputs["bd2"], f32),
        "bd3": np.asarray(inputs["bd3"], f32),
    }
    bos = _make_bos()
    latent = np.asarray(inputs["latent"], f32)
    target = np.asarray(inputs["target"], f32)
    in_maps = []
    for c in range(n_cores):
        sl = slice(c * bl, (c + 1) * bl)
        xT = np.empty((T, A, bl), np.float32)
        xT[0] = bos[sl].T
        if T > 1:
            xT[1:] = target[sl, 1:T].transpose(1, 2, 0)
        # [T, A, bl] -> [T, 32, 2, bl] DoubleRow ifmap layout (K-groups of 32)
        xT2 = np.ascontiguousarray(
            xT.reshape(T, 2, 32, bl).transpose(0, 2, 1, 3)).astype(FP8)
        m = dict(shared)
        m["latentT"] = np.ascontiguousarray(latent[sl].T).astype(BF16)
        m["xT"] = xT2
        in_maps.append(m)
    return in_maps


_NC_CACHE = {}


def _get_nc(steps=T, reps=1):
    key = (steps, reps)
    if key not in _NC_CACHE:
        _NC_CACHE[key] = _build(steps=steps, reps=reps)
    return _NC_CACHE[key]


def kernel(**inputs):
    nc = _get_nc()
    in_maps = _make_in_maps(inputs)
    res = bass_utils.run_bass_kernel_spmd(nc, in_maps,
                                          core_ids=list(range(N_CORES)))
    bl = B // N_CORES
    y = np.empty((B, L, A), np.float32)
    for c in range(N_CORES):
        y[c * bl:(c + 1) * bl] = res.results[c]["y"]
    y[:, 0, :] = _make_bos()
    return y


# revision 16
# speedup vs baseline: 1.2996x; 1.2547x over previous
"""Trainium2 Bass kernel for nn_Decoder (latent MLP -> GRU scan -> per-step MLP).

Strategy: pure data-parallel over batch (4096 -> 8 x 512), weights replicated.
Feature dim on partitions, batch on free dim. All recurrent matmuls (gi, gh,
wm1, wm2) run in fp8e4m3 with MatmulPerfMode.DoubleRow: each instruction
contracts TWO 128-row K-groups at 0.5 cycles/row (4x bf16 throughput). The
hidden state h is carried as fp8 "pair" tiles [128, 2, BL] matching the
DoubleRow ifmap layout. GRU tail: rhn = (hn+bhnn)*r is computed by DVE
in-place in the hn PSUM bank, then the x@W_ih n-gate matmul accumulates on
top (start=False), so tanh reads (gi_n + rhn) straight from PSUM and the
separate `s = inn + rhn` add disappears. d/zd run on Pool to offload DVE.

Self-contained: hardcodes shapes from the problem spec.
"""
import sys
sys.path.insert(0, "/opt/trn_rl_repo")
from contextlib import ExitStack

import numpy as np
import ml_dtypes

import concourse.bacc as bacc
import concourse.mybir as mybir
from concourse import tile
from concourse import bass_utils

BF16 = ml_dtypes.bfloat16
FP8 = ml_dtypes.float8_e4m3
BF = mybir.dt.bfloat16
F8 = mybir.dt.float8e4
F32 = mybir.dt.float32
AF = mybir.ActivationFunctionType
ALU = mybir.AluOpType
DR = mybir.MatmulPerfMode.DoubleRow

N_CORES = 8
B, LAT, H, A, L = 4096, 256, 512, 64, 128
BOS = 0
T = L - 1          # recurrence steps
BL = B // N_CORES  # per-core batch
KH = H // 128


def _build(steps=T, n_cores=N_CORES, reps=1, timing_iters=None, unroll=8,
           variant="full"):
    """Always declares the full-size DRAM interface (xT[T], y[:, L]); `steps`
    bounds the recurrence so short builds are wall-clock comparable.

    timing_iters: if set, wraps `unroll` statically-addressed step bodies in a
    hardware For_i loop executed timing_iters//unroll times (numerics garbage,
    per-step work identical) — used only to measure per-step device time."""
    nc = bacc.Bacc("TRN2", target_bir_lowering=False, debug=False,
                   num_devices=n_cores)

    d = {}
    def din(name, shape, dt=BF):
        d[name] = nc.dram_tensor(name, list(shape), dt, kind="ExternalInput").ap()

    din("latentT", [LAT, BL])
    din("xT", [T, 32, 2, BL], F8)
    din("Whh8_0", [128, 2, 3 * H], F8)
    din("Whh8_1", [128, 2, 3 * H], F8)
    din("Wih8", [32, 2, 3 * H], F8)
    din("Wm18_0", [128, 2, H], F8)
    din("Wm18_1", [128, 2, H], F8)
    din("Wm28_0", [128, 2, A], F8)
    din("Wm28_1", [128, 2, A], F8)
    din("Wm3T", [A, A])
    din("Wd1T", [LAT, H])
    din("Wd2T", [H, H])
    din("Wd3T", [H, H])
    din("b_rz", [2 * H], F32)
    din("b_inn", [H], F32)
    din("b_hnn", [H], F32)
    din("bm1", [H], F32)
    din("bm2", [A], F32)
    din("bm3b", [128, A], F32)
    din("bd1", [H], F32)
    din("bd2", [H], F32)
    din("bd3", [H], F32)
    y = nc.dram_tensor("y", [BL, L, A], F32, kind="ExternalOutput").ap()

    with tile.TileContext(nc) as tc, ExitStack() as ctx:
        cst = ctx.enter_context(tc.tile_pool(name="const", bufs=1))
        wrk = ctx.enter_context(tc.tile_pool(name="work", bufs=2))
        hpool = ctx.enter_context(tc.tile_pool(name="hp", bufs=4))
        # hn banks are long-lived (consumed by tanh-n at the END of the
        # step's tail): give them dedicated banks so the short-lived rz/pred
        # accs rotating in `ps` never wait on the n-chain.
        ps = ctx.enter_context(tc.tile_pool(name="ps", bufs=4, space="PSUM"))
        pshn = ctx.enter_context(tc.tile_pool(name="pshn", bufs=1,
                                              space="PSUM"))

        def const_tile(shape, dt, tag, src):
            t = cst.tile(list(shape), dt, tag=tag, name=tag)
            nc.sync.dma_start(t[:], src)
            return t

        whh8 = [const_tile([128, 2, 3 * H], F8, f"whh8{p}", d[f"Whh8_{p}"][:])
                for p in range(2)]
        wih8 = const_tile([32, 2, 3 * H], F8, "wih8", d["Wih8"][:])
        wm18 = [const_tile([128, 2, H], F8, f"wm18{p}", d[f"Wm18_{p}"][:])
                for p in range(2)]
        wm28 = [const_tile([128, 2, A], F8, f"wm28{p}", d[f"Wm28_{p}"][:])
                for p in range(2)]
        wm3 = const_tile([A, A], BF, "wm3", d["Wm3T"][:])
        wd1 = [const_tile([128, H], BF, f"wd1{k}",
                          d["Wd1T"][k * 128:(k + 1) * 128, :]) for k in range(2)]
        wd2 = [const_tile([128, H], BF, f"wd2{k}",
                          d["Wd2T"][k * 128:(k + 1) * 128, :]) for k in range(KH)]
        wd3 = [const_tile([128, H], BF, f"wd3{k}",
                          d["Wd3T"][k * 128:(k + 1) * 128, :]) for k in range(KH)]

        def bias_tiles(name, n, tag):
            return [const_tile([128, 1], F32, f"{tag}{j}",
                               d[name][j * 128:(j + 1) * 128, None])
                    for j in range(n)]

        brz = bias_tiles("b_rz", 8, "brz")
        binn = bias_tiles("b_inn", KH, "binn")
        bhnn = bias_tiles("b_hnn", KH, "bhnn")
        bm1 = bias_tiles("bm1", KH, "bm1")
        bm2 = const_tile([A, 1], F32, "bm2", d["bm2"][:, None])
        bm3b = const_tile([128, A], F32, "bm3b", d["bm3b"][:])
        bd1 = bias_tiles("bd1", KH, "bd1")
        bd2 = bias_tiles("bd2", KH, "bd2")
        bd3 = bias_tiles("bd3", KH, "bd3")

        lat = [const_tile([128, BL], BF, f"lat{k}",
                          d["latentT"][k * 128:(k + 1) * 128, :]) for k in range(2)]

        def mlp_layer(w_tiles, rhs_tiles, bias, act, out_tag, outs=None):
            ret = []
            for m in range(KH):
                acc = ps.tile([128, BL], F32, tag="ps", name="ps")
                nk = len(rhs_tiles)
                for k in range(nk):
                    nc.tensor.matmul(
                        acc[:], w_tiles[k][:, m * 128:(m + 1) * 128],
                        rhs_tiles[k][:], start=(k == 0), stop=(k == nk - 1))
                if outs is None:
                    o = wrk.tile([128, BL], BF, tag=f"{out_tag}{m}",
                                 name=f"{out_tag}{m}")
                    nc.scalar.activation(o[:], acc[:], act, bias=bias[m][:])
                    ret.append(o)
                else:
                    nc.scalar.activation(outs[m], acc[:], act, bias=bias[m][:])
            return ret

        h1 = mlp_layer(wd1, lat, bd1, AF.Tanh, "h1")
        h2 = mlp_layer(wd2, h1, bd2, AF.Tanh, "h2")
        # final init layer writes fp8 pair tiles directly
        hp0 = [hpool.tile([128, 2, BL], F8, tag=f"hb{p}", name=f"hb{p}")
               for p in range(2)]
        mlp_layer(wd3, h2, bd3, AF.Identity, "hb",
                  outs=[hp0[m // 2][:, m % 2, :] for m in range(KH)])

        # hist[t] = h pair tiles of step t (init state = hist[-1]); pred for
        # step t is emitted 2 steps later so its matmuls fill the PE stall
        # while the h(t-1) elementwise tail completes.
        state = {"hist": {-1: hp0}, "ystage": None}

        def gh_mm(acc, m, hbp, start, stop):
            # DoubleRow gh: 2 instructions contract all 512 rows of h
            for p in range(2):
                nc.tensor.matmul(
                    acc[:], whh8[p][:, :, m * 128:(m + 1) * 128],
                    hbp[p][:, :, :], start=(start and p == 0),
                    stop=(stop and p == 1), perf_mode=DR)

        def gi_mm(acc, m, xt, start, stop, skip=False):
            nc.tensor.matmul(acc[:], wih8[:, :, m * 128:(m + 1) * 128],
                             xt[:, :, :], start=start, stop=stop, perf_mode=DR,
                             skip_group_check=skip)

        want_act = variant in ("full", "nopred", "notail")
        want_tail = variant in ("full", "nopred")
        want_pred_tail = variant == "full"

        def emit_gates(t, force_pred=False):
            # Critical-path-first schedule: the recurrence loop is
            # h'(t-1) -> r-accs -> sig r -> rhn -> gi_n -> tanh n -> d ->
            # zd -> h'(t). Emit exactly that chain first; z-gates and
            # pred(t-2) are off-path and go last as shadow work that keeps
            # PE/ACT busy while the tail drains and the next step starts.
            hbp = state["hist"][t - 1]
            xt = wrk.tile([32, 2, BL], F8, tag="xt", name="xt")
            nc.sync.dma_start(xt[:], d["xT"][t])

            def rz_mm(m0):
                accs = []
                for m in (m0, m0 + 1):
                    acc = ps.tile([128, BL], F32, tag="ps", name="ps")
                    gi_mm(acc, m, xt, start=True, stop=False)
                    accs.append(acc)
                for i, m in enumerate((m0, m0 + 1)):
                    gh_mm(accs[i], m, hbp, start=False, stop=True)
                return accs

            def rz_sig(m0, accs):
                out = []
                if want_act:
                    for i, m in enumerate((m0, m0 + 1)):
                        g = wrk.tile([128, BL], BF, tag=f"rz{m}",
                                     name=f"rz{m}")
                        nc.scalar.activation(g[:], accs[i][:], AF.Sigmoid,
                                             bias=brz[m][:])
                        out.append(g)
                return out

            def rz_pair(m0):
                return rz_sig(m0, rz_mm(m0))

            r = rz_pair(0) + rz_pair(2)

            hn_ps = []
            for j in range(KH):
                hn = pshn.tile([128, BL], F32, tag=f"hn{j}", name=f"hn{j}")
                gh_mm(hn, 8 + j, hbp, start=True, stop=True)
                hn_ps.append(hn)

            if not want_tail:
                rz_pair(4), rz_pair(6)
                if want_act:
                    for j in range(KH):
                        gi_mm(hn_ps[j], 8 + j, xt, start=False, stop=True,
                              skip=True)
                        n_t = wrk.tile([128, BL], BF, tag=f"nt{j}",
                                       name=f"nt{j}")
                        nc.scalar.activation(n_t[:], hn_ps[j][:], AF.Tanh,
                                             bias=binn[j][:])
                state["hist"][t] = hbp
                emit_pred(t - 2, last=False, force=force_pred)
                return

            for j in range(KH):
                # rhn = (hn + bhnn) * r, written back into the same PSUM bank
                nc.vector.scalar_tensor_tensor(
                    hn_ps[j][:], hn_ps[j][:], bhnn[j][:], r[j][:],
                    op0=ALU.add, op1=ALU.mult)
            # z-gate MATMULS here: PE fill during the rhn wait (their
            # sigmoids stay late in the ACT queue so tanh-n isn't delayed)
            zacc0, zacc1 = rz_mm(4), rz_mm(6)
            for j in range(KH):
                # accumulate x @ W_ih_n on top of rhn
                gi_mm(hn_ps[j], 8 + j, xt, start=False, stop=True, skip=True)
            n_ts = []
            for j in range(KH):
                n_t = wrk.tile([128, BL], BF, tag=f"nt{j}", name=f"nt{j}")
                nc.scalar.activation(n_t[:], hn_ps[j][:], AF.Tanh,
                                     bias=binn[j][:])
                n_ts.append(n_t)
            z = rz_sig(4, zacc0) + rz_sig(6, zacc1)

            hb_new = [hpool.tile([128, 2, BL], F8, tag=f"hb{p}", name=f"hb{p}")
                      for p in range(2)]
            d_ts = []
            for j in range(KH):
                d_t = wrk.tile([128, BL], BF, tag=f"dt{j}", name=f"dt{j}")
                nc.gpsimd.tensor_sub(d_t[:], hbp[j // 2][:, j % 2, :],
                                     n_ts[j][:])
                d_ts.append(d_t)
            zds = []
            for j in range(KH):
                zd = wrk.tile([128, BL], BF, tag=f"zd{j}", name=f"zd{j}")
                nc.vector.tensor_mul(zd[:], z[j][:], d_ts[j][:])
                zds.append(zd)
            for j in range(KH):
                nc.vector.tensor_add(hb_new[j // 2][:, j % 2, :], n_ts[j][:],
                                     zds[j][:])
            state["hist"][t] = hb_new

            # pred(t-2) last: PE/ACT shadow work under the tail and the
            # next step's r-acc phase
            emit_pred(t - 2, last=False, force=force_pred)

        def emit_pred(tp, last, force=False):
            if tp < 0 and not force:
                return
            hbp = (state["hist"][tp] if tp in state["hist"]
                   else state["hist"][-1])
            ystage = state["ystage"]
            p1p = [wrk.tile([128, 2, BL], F8, tag=f"p1p{p}", name=f"p1p{p}")
                   for p in range(2)]
            for m in range(KH):
                acc = ps.tile([128, BL], F32, tag="ps", name="ps")
                for p in range(2):
                    nc.tensor.matmul(
                        acc[:], wm18[p][:, :, m * 128:(m + 1) * 128],
                        hbp[p][:, :, :], start=(p == 0), stop=(p == 1),
                        perf_mode=DR)
                if want_pred_tail:
                    nc.scalar.activation(p1p[m // 2][:, m % 2, :], acc[:],
                                         AF.Tanh, bias=bm1[m][:])
            if not want_pred_tail:
                state["hist"].pop(tp - 1, None)
                return
            acc2 = ps.tile([A, BL], F32, tag="ps", name="ps")
            for p in range(2):
                nc.tensor.matmul(acc2[:], wm28[p][:, :, :], p1p[p][:, :, :],
                                 start=(p == 0), stop=(p == 1), perf_mode=DR,
                                 skip_group_check=True)
            p2 = wrk.tile([A, BL], BF, tag="p2", name="p2")
            nc.scalar.activation(p2[:], acc2[:], AF.Tanh, bias=bm2[:])

            tps = tp if tp >= 0 else tp + 8  # timing-build pseudo-slot
            o = (tps + 1) % 8
            g = (tps + 1) // 8
            if ystage is None or o == 0 or (g == 0 and o == 1):
                ystage = [wrk.tile([128, 8 * A], F32, tag=f"yst{bt}",
                                   name=f"yst{bt}") for bt in range(4)]
            for bt in range(4):
                yp = ps.tile([128, A], F32, tag="ps", name="ps")
                nc.tensor.matmul(yp[:], p2[:, bt * 128:(bt + 1) * 128],
                                 wm3[:], start=True, stop=True)
                nc.vector.tensor_add(
                    ystage[bt][:, o * A:(o + 1) * A], yp[:], bm3b[:])
            if o == 7 or last:
                lo = 1 if g == 0 else 0
                hi = o + 1
                for bt in range(4):
                    nc.sync.dma_start(
                        y[bt * 128:(bt + 1) * 128, g * 8 + lo:g * 8 + hi, :],
                        ystage[bt][:, lo * A:hi * A])
            state["ystage"] = ystage
            # h(tp) no longer needed once its pred is done
            state["hist"].pop(tp - 1, None)

        PRED_LAG = 2
        if timing_iters is None:
            for _rep in range(reps):
                for t in range(steps):
                    emit_gates(t)
                for tp in range(max(steps - PRED_LAG, 0), steps):
                    emit_pred(tp, last=(tp == steps - 1))
        else:
            # timing loop: same per-step work (preds for t<LAG read init h —
            # numerics are garbage in timing builds anyway)
            with tc.For_i(0, timing_iters // unroll, 1):
                for t in range(unroll):
                    emit_gates(t, force_pred=True)

    nc.compile()
    return nc


def _make_bos():
    bos = np.full((B, A), -16.0, np.float32)
    bos[:, BOS] = 16.0
    return bos


def _pair(mT):
    """[2K, M] -> list of 2 fp8 pair tensors [128, 2, M] for DoubleRow."""
    K = mT.shape[0]
    assert K == 512
    return [np.ascontiguousarray(
        np.stack([mT[(2 * p) * 128:(2 * p + 1) * 128],
                  mT[(2 * p + 1) * 128:(2 * p + 2) * 128]], axis=1)
    ).astype(FP8) for p in range(2)]


def _make_in_maps(inputs, n_cores=N_CORES, T=T):
    bl = B // n_cores
    f32 = np.float32
    whhT = np.ascontiguousarray(np.asarray(inputs["W_hh"], f32).T)
    wm1T = np.ascontiguousarray(np.asarray(inputs["Wm1"], f32).T)
    wm2T = np.ascontiguousarray(np.asarray(inputs["Wm2"], f32).T)
    wihT = np.ascontiguousarray(np.asarray(inputs["W_ih"], f32).T)  # [64,3H]
    whh8 = _pair(whhT)
    wm18 = _pair(wm1T)
    wm28 = _pair(wm2T)
    wih8 = np.ascontiguousarray(
        wihT.reshape(2, 32, 3 * H).transpose(1, 0, 2)).astype(FP8)
    shared = {
        "Whh8_0": whh8[0], "Whh8_1": whh8[1],
        "Wm18_0": wm18[0], "Wm18_1": wm18[1],
        "Wm28_0": wm28[0], "Wm28_1": wm28[1],
        "Wih8": wih8,
        "Wm3T": np.ascontiguousarray(np.asarray(inputs["Wm3"], f32).T).astype(BF16),
        "Wd1T": np.ascontiguousarray(np.asarray(inputs["Wd1"], f32).T).astype(BF16),
        "Wd2T": np.ascontiguousarray(np.asarray(inputs["Wd2"], f32).T).astype(BF16),
        "Wd3T": np.ascontiguousarray(np.asarray(inputs["Wd3"], f32).T).astype(BF16),
        "b_rz": (np.asarray(inputs["b_ih"], f32)
                 + np.asarray(inputs["b_hh"], f32))[:2 * H].astype(f32),
        "b_inn": np.asarray(inputs["b_ih"], f32)[2 * H:].astype(f32),
        "b_hnn": np.asarray(inputs["b_hh"], f32)[2 * H:].astype(f32),
        "bm1": np.asarray(inputs["bm1"], f32),
        "bm2": np.asarray(inputs["bm2"], f32),
        "bm3b": np.ascontiguousarray(
            np.broadcast_to(np.asarray(inputs["bm3"], f32), (128, A))),
        "bd1": np.asarray(inputs["bd1"], f32),
        "bd2": np.asarray(inputs["bd2"], f32),
        "bd3": np.asarray(inputs["bd3"], f32),
    }
    bos = _make_bos()
    latent = np.asarray(inputs["latent"], f32)
    target = np.asarray(inputs["target"], f32)
    in_maps = []
    for c in range(n_cores):
        sl = slice(c * bl, (c + 1) * bl)
        xT = np.empty((T, A, bl), np.float32)
        xT[0] = bos[sl].T
        if T > 1:
            xT[1:] = target[sl, 1:T].transpose(1, 2, 0)
        # [T, A, bl] -> [T, 32, 2, bl] DoubleRow ifmap layout (K-groups of 32)
        xT2 = np.ascontiguousarray(
            xT.reshape(T, 2, 32, bl).transpose(0, 2, 1, 3)).astype(FP8)
        m = dict(shared)
        m["latentT"] = np.ascontiguousarray(latent[sl].T).astype(BF16)
        m["xT"] = xT2
        in_maps.append(m)
    return in_maps


_NC_CACHE = {}


def _get_nc(steps=T, reps=1):
    key = (steps, reps)
    if key not in _NC_CACHE:
        _NC_CACHE[key] = _build(steps=steps, reps=reps)
    return _NC_CACHE[key]


def kernel(**inputs):
    nc = _get_nc()
    in_maps = _make_in_maps(inputs)
    res = bass_utils.run_bass_kernel_spmd(nc, in_maps,
                                          core_ids=list(range(N_CORES)))
    bl = B // N_CORES
    y = np.empty((B, L, A), np.float32)
    for c in range(N_CORES):
        y[c * bl:(c + 1) * bl] = res.results[c]["y"]
    y[:, 0, :] = _make_bos()
    return y
